# revision 1
# baseline (speedup 1.0000x reference)
"""Neighbor aggregation (GNN message passing) on 8 Trainium2 cores.

out[b, u] = sum_{edges (u, v) in batch b} w_e * H[b, v]    (messages are 16x16 blocks)

Sharding: core (b, h) handles batch b's edges whose destination falls in
dst-half h (h=0: nodes [0, 25088), h=1: [25088, 50048)).  Outputs are disjoint
-> no cross-core reduction.  Within a core, edges are bucketed by 128-node dst
block and by src half (so gather indices fit in int16), padded to a uniform
number of 128-edge groups per bucket.  Device loop per dst block:
  dma_gather 128-row groups of H   (SWDGE bulk gather)
  one-hot weight matrix W[e, d] = w_e * (d == dstloc_e)   (one DVE op)
  PSUM += W.T @ Hgather                                    (fp32 matmul)
  copy PSUM -> SBUF -> DRAM out rows.
"""

import sys

sys.path.insert(0, "/opt/trn_rl_repo")

import numpy as np

import concourse.bacc as bacc
import concourse.tile as tile
from concourse import mybir
from concourse.bass_utils import run_bass_kernel_spmd

B = 4
N_NODES = 50000
HS = 16
C = HS * HS          # 256 floats per message row
P = 128
E = 800000
NBLK = 196           # dst blocks per core (196*128 = 25088 rows of output)
HALF0 = NBLK * P     # dst boundary between the two cores of a batch
SRC_SPLIT = 25000    # src half boundary; local indices stay < 32768 (int16)
NSEG = NBLK * 2      # (block, src-half) buckets per core
N_CORES = 8

_prog_cache: dict[int, object] = {}
_last_in_maps: list | None = None


def _build_program(gh: int):
    """Bass program for all 8 cores; gh = 128-edge groups per (block, src-half)."""
    ngrp = NSEG * gh             # one-hot groups per core
    idx_cols = NSEG * gh * 8     # int16 idx columns (16 idx per column)

    nc = bacc.Bacc("TRN2", target_bir_lowering=False, debug=False)
    h_d = nc.dram_tensor("h", (N_NODES, C), mybir.dt.float32, kind="ExternalInput")
    idx_d = nc.dram_tensor("idx", (P, idx_cols), mybir.dt.int16, kind="ExternalInput")
    mdst_d = nc.dram_tensor("mdst", (P, ngrp), mybir.dt.float32, kind="ExternalInput")
    mw_d = nc.dram_tensor("mw", (P, ngrp), mybir.dt.float32, kind="ExternalInput")
    iota_d = nc.dram_tensor("iota", (P, P), mybir.dt.float32, kind="ExternalInput")
    out_d = nc.dram_tensor("out", (NBLK * P, C), mybir.dt.float32, kind="ExternalOutput")

    h_ap = h_d.ap()
    src_half_aps = (h_ap[0:SRC_SPLIT, :], h_ap[SRC_SPLIT:N_NODES, :])

    with tile.TileContext(nc) as tc:
        with tc.tile_pool(name="const", bufs=1) as cpool, \
             tc.tile_pool(name="gat", bufs=8) as gpool, \
             tc.tile_pool(name="wtile", bufs=4) as wpool, \
             tc.tile_pool(name="otile", bufs=4) as opool, \
             tc.tile_pool(name="psum", bufs=8, space="PSUM") as ppool:
            iota_t = cpool.tile([P, P], mybir.dt.float32)
            nc.sync.dma_start(out=iota_t[:], in_=iota_d.ap())
            mdst_t = cpool.tile([P, ngrp], mybir.dt.float32)
            nc.sync.dma_start(out=mdst_t[:], in_=mdst_d.ap())
            mw_t = cpool.tile([P, ngrp], mybir.dt.float32)
            nc.sync.dma_start(out=mw_t[:], in_=mw_d.ap())
            idx_t = cpool.tile([P, idx_cols], mybir.dt.int16)
            nc.sync.dma_start(out=idx_t[:], in_=idx_d.ap())

            for j in range(NBLK):
                gt = []
                for s in range(2):
                    seg = j * 2 + s
                    # dma_gather tops out at 1024 indices per call; separate
                    # tiles per chunk so the calls don't serialize on WAW
                    parts = []
                    for ci, c0 in enumerate(range(0, gh, 8)):
                        c1 = min(c0 + 8, gh)
                        g = gpool.tile([P, c1 - c0, C], mybir.dt.float32,
                                       tag=f"gat{ci}")
                        nc.gpsimd.dma_gather(
                            out_ap=g[:],
                            in_ap=src_half_aps[s],
                            idxs_ap=idx_t[:, seg * gh * 8 + c0 * 8:seg * gh * 8 + c1 * 8],
                            num_idxs=(c1 - c0) * P,
                            num_idxs_reg=(c1 - c0) * P,
                            elem_size=C,
                        )
                        parts.append((g, c0, c1))
                    gt.append(parts)

                acc = ppool.tile([P, C], mybir.dt.float32, space="PSUM")
                for gi in range(2 * gh):
                    s, gg = divmod(gi, gh)
                    col = (j * 2 + s) * gh + gg
                    g, c0, _ = next(p for p in gt[s] if p[1] <= gg < p[2])
                    W = wpool.tile([P, P], mybir.dt.float32, tag="W")
                    nc.vector.tensor_scalar(
                        out=W[:],
                        in0=iota_t[:],
                        scalar1=mdst_t[:, col:col + 1],
                        scalar2=mw_t[:, col:col + 1],
                        op0=mybir.AluOpType.is_equal,
                        op1=mybir.AluOpType.mult,
                    )
                    nc.tensor.matmul(
                        out=acc[:],
                        lhsT=W[:],
                        rhs=g[:, gg - c0, :],
                        start=(gi == 0),
                        stop=(gi == 2 * gh - 1),
                    )
                ot = opool.tile([P, C], mybir.dt.float32, tag="out")
                nc.any.tensor_copy(out=ot[:], in_=acc[:])
                nc.sync.dma_start(out=out_d.ap()[j * P:(j + 1) * P, :], in_=ot[:])

    nc.compile()
    return nc


def kernel(H, edge_index, edge_weight, node_idx):
    H = np.ascontiguousarray(np.asarray(H), dtype=np.float32)
    edge_index = np.asarray(edge_index)
    edge_weight = np.ascontiguousarray(np.asarray(edge_weight), dtype=np.float32)
    node_idx = np.asarray(node_idx)

    inv = np.argsort(node_idx).astype(np.int64)  # id -> row (identity for arange)
    iota = np.tile(np.arange(P, dtype=np.float32), (P, 1))

    # ---- host bucketing: (core, dst-block, src-half) ----
    per_core = []   # (sloc_sorted, dloc_sorted, w_sorted, counts) per core
    gh = 1
    for b in range(B):
        dst = inv[edge_index[b, :, 0]]
        src = inv[edge_index[b, :, 1]]
        w = edge_weight[b]
        half = dst >= HALF0
        for h in (0, 1):
            m = half == (h == 1)
            d = dst[m] - h * HALF0
            s_rows = src[m]
            sh = s_rows >= SRC_SPLIT
            sloc = (s_rows - sh * SRC_SPLIT).astype(np.int16)
            bucket = (d >> 7) * 2 + sh
            order = np.argsort(bucket, kind="stable")
            bs = bucket[order]
            counts = np.bincount(bs, minlength=NSEG)
            gh = max(gh, int(np.ceil(counts.max() / P)))
            per_core.append((sloc[order], (d & 127)[order].astype(np.float32),
                             w[m][order], bs, counts))

    ngrp = NSEG * gh
    slots = ngrp * P
    in_maps = []
    for core in range(N_CORES):
        sloc, dloc, wv, bs, counts = per_core[core]
        starts = np.zeros(NSEG, np.int64)
        starts[1:] = np.cumsum(counts)[:-1]
        rank = np.arange(len(bs)) - starts[bs]
        slot = bs.astype(np.int64) * (gh * P) + rank

        sl = np.zeros(slots, np.int16)  # pads gather row 0 with w=0
        dl = np.zeros(slots, np.float32)
        wl = np.zeros(slots, np.float32)
        sl[slot] = sloc
        dl[slot] = dloc
        wl[slot] = wv

        # idx element e of segment k -> [e % 16, k*gh*8 + e//16], replicated x8
        idx16 = sl.reshape(NSEG, gh * 8, 16).transpose(2, 0, 1).reshape(16, NSEG * gh * 8)
        idx128 = np.ascontiguousarray(np.tile(idx16, (8, 1)))
        mdst = np.ascontiguousarray(dl.reshape(ngrp, P).T)
        mw = np.ascontiguousarray(wl.reshape(ngrp, P).T)

        in_maps.append({
            "h": H[core // 2].reshape(N_NODES, C),
            "idx": idx128,
            "mdst": mdst,
            "mw": mw,
            "iota": iota,
        })

    global _last_in_maps
    _last_in_maps = in_maps
    nc = _prog_cache.get(gh)
    if nc is None:
        nc = _build_program(gh)
        _prog_cache[gh] = nc

    res = run_bass_kernel_spmd(nc, in_maps, list(range(N_CORES)))

    out = np.empty((B, N_NODES, HS, HS), np.float32)
    for b in range(B):
        r0 = res.results[2 * b]["out"]
        r1 = res.results[2 * b + 1]["out"]
        out[b, :HALF0] = r0.reshape(-1, HS, HS)
        out[b, HALF0:] = r1[:N_NODES - HALF0].reshape(-1, HS, HS)
    return out



# revision 3
# speedup vs baseline: 1.0880x; 1.0880x over previous
"""Neighbor aggregation (GNN message passing) on 8 Trainium2 cores.

out[b, u] = sum_{edges (u, v) in batch b} w_e * H[b, v]    (messages are 16x16 blocks)

Sharding: core (b, h) handles batch b's edges whose destination falls in
dst-half h (h=0: nodes [0, 25088), h=1: [25088, 50048)).  Outputs are disjoint
-> no cross-core reduction.  Within a core, edges are bucketed by 128-node dst
block and by src half (so gather indices fit in int16), padded to a uniform
number of 128-edge groups per bucket.  Device loop per dst block:
  dma_gather 128-row groups of H (bf16, SWDGE) -- calls rotate across the 4
  SWDGE queues so descriptor generation overlaps across Q7 CPU pairs
  one-hot weight matrix W[e, d] = w_e * (d == dstloc_e)   (one DVE op, bf16)
  PSUM += W.T @ Hgather                                    (bf16 matmul)
  copy PSUM -> SBUF -> DRAM out rows (fp32).
"""

import sys

sys.path.insert(0, "/opt/trn_rl_repo")

import numpy as np
import ml_dtypes

import concourse.bacc as bacc
import concourse.tile as tile
from concourse import mybir
from concourse.bass_utils import run_bass_kernel_spmd

B = 4
N_NODES = 50000
HS = 16
C = HS * HS          # 256 values per message row
P = 128
E = 800000
NBLK = 196           # dst blocks per core (196*128 = 25088 rows of output)
HALF0 = NBLK * P     # dst boundary between the two cores of a batch
SRC_SPLIT = 25000    # src half boundary; local indices stay < 32768 (int16)
NSEG = NBLK * 2      # (block, src-half) buckets per core
N_CORES = 8
NQ = 4               # SWDGE queues (gather descriptor-gen pipelines)

_prog_cache: dict[int, object] = {}
_last_in_maps: list | None = None


def _build_program(gh: int):
    """Bass program for all 8 cores; gh = 128-edge groups per (block, src-half)."""
    ngrp = NSEG * gh             # one-hot groups per core
    idx_cols = NSEG * gh * 8     # int16 idx columns (16 idx per column)

    nc = bacc.Bacc("TRN2", target_bir_lowering=False, debug=False,
                   num_swdge_queues=NQ)
    h_d = nc.dram_tensor("h", (N_NODES, C), mybir.dt.bfloat16, kind="ExternalInput")
    idx_d = nc.dram_tensor("idx", (P, idx_cols), mybir.dt.int16, kind="ExternalInput")
    mdst_d = nc.dram_tensor("mdst", (P, ngrp), mybir.dt.float32, kind="ExternalInput")
    mw_d = nc.dram_tensor("mw", (P, ngrp), mybir.dt.float32, kind="ExternalInput")
    iota_d = nc.dram_tensor("iota", (P, P), mybir.dt.bfloat16, kind="ExternalInput")
    out_d = nc.dram_tensor("out", (NBLK * P, C), mybir.dt.float32, kind="ExternalOutput")

    h_ap = h_d.ap()
    src_half_aps = (h_ap[0:SRC_SPLIT, :], h_ap[SRC_SPLIT:N_NODES, :])

    qctr = 0  # rotates gather calls over the SWDGE queues

    with tile.TileContext(nc) as tc:
        with tc.tile_pool(name="const", bufs=1) as cpool, \
             tc.tile_pool(name="gat", bufs=12) as gpool, \
             tc.tile_pool(name="wtile", bufs=4) as wpool, \
             tc.tile_pool(name="otile", bufs=4) as opool, \
             tc.tile_pool(name="psum", bufs=8, space="PSUM") as ppool:
            iota_t = cpool.tile([P, P], mybir.dt.bfloat16)
            nc.sync.dma_start(out=iota_t[:], in_=iota_d.ap())
            mdst_t = cpool.tile([P, ngrp], mybir.dt.float32)
            nc.sync.dma_start(out=mdst_t[:], in_=mdst_d.ap())
            mw_t = cpool.tile([P, ngrp], mybir.dt.float32)
            nc.sync.dma_start(out=mw_t[:], in_=mw_d.ap())
            idx_t = cpool.tile([P, idx_cols], mybir.dt.int16)
            nc.sync.dma_start(out=idx_t[:], in_=idx_d.ap())

            for j in range(NBLK):
                gt = []
                for s in range(2):
                    seg = j * 2 + s
                    # dma_gather tops out at 1024 indices per call; separate
                    # tiles per chunk so the calls don't serialize on WAW
                    parts = []
                    for ci, c0 in enumerate(range(0, gh, 8)):
                        c1 = min(c0 + 8, gh)
                        g = gpool.tile([P, c1 - c0, C], mybir.dt.bfloat16,
                                       tag=f"gat{ci}")
                        nc.gpsimd.dma_gather(
                            out_ap=g[:],
                            in_ap=src_half_aps[s],
                            idxs_ap=idx_t[:, seg * gh * 8 + c0 * 8:seg * gh * 8 + c1 * 8],
                            num_idxs=(c1 - c0) * P,
                            num_idxs_reg=(c1 - c0) * P,
                            elem_size=C,
                            queue_num=qctr % NQ,
                        )
                        qctr += 1
                        parts.append((g, c0, c1))
                    gt.append(parts)

                acc = ppool.tile([P, C], mybir.dt.float32, space="PSUM")
                for gi in range(2 * gh):
                    s, gg = divmod(gi, gh)
                    col = (j * 2 + s) * gh + gg
                    g, c0, _ = next(p for p in gt[s] if p[1] <= gg < p[2])
                    W = wpool.tile([P, P], mybir.dt.bfloat16, tag="W")
                    nc.vector.tensor_scalar(
                        out=W[:],
                        in0=iota_t[:],
                        scalar1=mdst_t[:, col:col + 1],
                        scalar2=mw_t[:, col:col + 1],
                        op0=mybir.AluOpType.is_equal,
                        op1=mybir.AluOpType.mult,
                    )
                    nc.tensor.matmul(
                        out=acc[:],
                        lhsT=W[:],
                        rhs=g[:, gg - c0, :],
                        start=(gi == 0),
                        stop=(gi == 2 * gh - 1),
                    )
                ot = opool.tile([P, C], mybir.dt.float32, tag="out")
                nc.any.tensor_copy(out=ot[:], in_=acc[:])
                nc.sync.dma_start(out=out_d.ap()[j * P:(j + 1) * P, :], in_=ot[:])

    nc.compile()
    return nc


def kernel(H, edge_index, edge_weight, node_idx):
    H = np.asarray(H)
    edge_index = np.asarray(edge_index)
    edge_weight = np.ascontiguousarray(np.asarray(edge_weight), dtype=np.float32)
    node_idx = np.asarray(node_idx)

    inv = np.argsort(node_idx).astype(np.int64)  # id -> row (identity for arange)
    iota = np.tile(np.arange(P, dtype=np.float32), (P, 1)).astype(ml_dtypes.bfloat16)

    # ---- host bucketing: (core, dst-block, src-half) ----
    per_core = []   # (sloc_sorted, dloc_sorted, w_sorted, counts) per core
    gh = 1
    for b in range(B):
        dst = inv[edge_index[b, :, 0]]
        src = inv[edge_index[b, :, 1]]
        w = edge_weight[b]
        half = dst >= HALF0
        for h in (0, 1):
            m = half == (h == 1)
            d = dst[m] - h * HALF0
            s_rows = src[m]
            sh = s_rows >= SRC_SPLIT
            sloc = (s_rows - sh * SRC_SPLIT).astype(np.int16)
            bucket = (d >> 7) * 2 + sh
            order = np.argsort(bucket, kind="stable")
            bs = bucket[order]
            counts = np.bincount(bs, minlength=NSEG)
            gh = max(gh, int(np.ceil(counts.max() / P)))
            per_core.append((sloc[order], (d & 127)[order].astype(np.float32),
                             w[m][order], bs, counts))

    ngrp = NSEG * gh
    slots = ngrp * P
    in_maps = []
    for core in range(N_CORES):
        sloc, dloc, wv, bs, counts = per_core[core]
        starts = np.zeros(NSEG, np.int64)
        starts[1:] = np.cumsum(counts)[:-1]
        rank = np.arange(len(bs)) - starts[bs]
        slot = bs.astype(np.int64) * (gh * P) + rank

        sl = np.zeros(slots, np.int16)  # pads gather row 0 with w=0
        dl = np.zeros(slots, np.float32)
        wl = np.zeros(slots, np.float32)
        sl[slot] = sloc
        dl[slot] = dloc
        wl[slot] = wv

        # idx element e of segment k -> [e % 16, k*gh*8 + e//16], replicated x8
        idx16 = sl.reshape(NSEG, gh * 8, 16).transpose(2, 0, 1).reshape(16, NSEG * gh * 8)
        idx128 = np.ascontiguousarray(np.tile(idx16, (8, 1)))
        mdst = np.ascontiguousarray(dl.reshape(ngrp, P).T)
        mw = np.ascontiguousarray(wl.reshape(ngrp, P).T)

        in_maps.append({
            "h": np.ascontiguousarray(H[core // 2].reshape(N_NODES, C)).astype(
                ml_dtypes.bfloat16),
            "idx": idx128,
            "mdst": mdst,
            "mw": mw,
            "iota": iota,
        })

    global _last_in_maps
    _last_in_maps = in_maps
    nc = _prog_cache.get(gh)
    if nc is None:
        nc = _build_program(gh)
        _prog_cache[gh] = nc

    res = run_bass_kernel_spmd(nc, in_maps, list(range(N_CORES)))

    out = np.empty((B, N_NODES, HS, HS), np.float32)
    for b in range(B):
        r0 = res.results[2 * b]["out"]
        r1 = res.results[2 * b + 1]["out"]
        out[b, :HALF0] = r0.reshape(-1, HS, HS)
        out[b, HALF0:] = r1[:N_NODES - HALF0].reshape(-1, HS, HS)
    return out


# revision 4
# speedup vs baseline: 2.8898x; 2.6561x over previous
"""Neighbor aggregation (GNN message passing) on 8 Trainium2 cores.

out[b, u] = sum_{edges (u, v) in batch b} w_e * H[b, v]    (messages are 16x16 blocks)

Sharding: core (b, h) handles batch b's edges whose destination falls in
dst-half h (h=0: nodes [0, 25088), h=1: [25088, 50048)).  Outputs are disjoint
-> no cross-core reduction.  Within a core, edges are bucketed by 128-node dst
block and by src half (so gather indices fit in int16), padded to a uniform
number of 128-edge groups per bucket.  Device loop per dst block:
  dma_gather 128-row groups of H (bf16, SWDGE) -- calls balanced across the 4
  SWDGE queues so descriptor generation overlaps across Q7 CPU pairs
  one-hot weight matrices W[e, d] = w_e * (d == dstloc_e) are precomputed on
  the host and streamed per block (one dma_start per block, no DVE work)
  PSUM += W.T @ Hgather                                    (bf16 matmul)
  copy PSUM -> SBUF -> DRAM out rows (fp32).
"""

import sys

sys.path.insert(0, "/opt/trn_rl_repo")

import numpy as np
import ml_dtypes

import concourse.bacc as bacc
import concourse.tile as tile
from concourse import mybir
from concourse.bass_utils import run_bass_kernel_spmd

B = 4
N_NODES = 50000
HS = 16
C = HS * HS          # 256 values per message row
P = 128
E = 800000
NBLK = 196           # dst blocks per core (196*128 = 25088 rows of output)
HALF0 = NBLK * P     # dst boundary between the two cores of a batch
SRC_SPLIT = 25000    # src half boundary; local indices stay < 32768 (int16)
NSEG = NBLK * 2      # (block, src-half) buckets per core
N_CORES = 8
NQ = 4               # SWDGE queues (gather descriptor-gen pipelines)

_prog_cache: dict[int, object] = {}
_last_in_maps: list | None = None


def _build_program(gh: int):
    """Bass program for all 8 cores; gh = 128-edge groups per (block, src-half)."""
    ngrp = NSEG * gh             # one-hot groups per core
    idx_cols = NSEG * gh * 8     # int16 idx columns (16 idx per column)

    nc = bacc.Bacc("TRN2", target_bir_lowering=False, debug=False,
                   num_swdge_queues=NQ)
    h_d = nc.dram_tensor("h", (N_NODES, C), mybir.dt.bfloat16, kind="ExternalInput")
    idx_d = nc.dram_tensor("idx", (P, idx_cols), mybir.dt.int16, kind="ExternalInput")
    w_d = nc.dram_tensor("w", (P, ngrp * P), mybir.dt.bfloat16, kind="ExternalInput")
    out_d = nc.dram_tensor("out", (NBLK * P, C), mybir.dt.float32, kind="ExternalOutput")

    h_ap = h_d.ap()
    src_half_aps = (h_ap[0:SRC_SPLIT, :], h_ap[SRC_SPLIT:N_NODES, :])

    qbig = 0    # queue rotation, big gather calls
    qsmall = 1  # queue rotation, small gather calls (offset to interleave)

    with tile.TileContext(nc) as tc:
        with tc.tile_pool(name="const", bufs=1) as cpool, \
             tc.tile_pool(name="gat", bufs=12) as gpool, \
             tc.tile_pool(name="wtile", bufs=4) as wpool, \
             tc.tile_pool(name="otile", bufs=4) as opool, \
             tc.tile_pool(name="psum", bufs=8, space="PSUM") as ppool:
            idx_t = cpool.tile([P, idx_cols], mybir.dt.int16)
            nc.sync.dma_start(out=idx_t[:], in_=idx_d.ap())

            for j in range(NBLK):
                wt = wpool.tile([P, 2 * gh * P], mybir.dt.bfloat16, tag="W")
                nc.sync.dma_start(
                    out=wt[:],
                    in_=w_d.ap()[:, j * 2 * gh * P:(j + 1) * 2 * gh * P])

                gt = []
                for s in range(2):
                    seg = j * 2 + s
                    # dma_gather tops out at 1024 indices per call; separate
                    # tiles per chunk so the calls don't serialize on WAW
                    parts = []
                    for ci, c0 in enumerate(range(0, gh, 8)):
                        c1 = min(c0 + 8, gh)
                        if c1 - c0 == 8:
                            q = qbig % NQ
                            qbig += 1
                        else:
                            q = qsmall % NQ
                            qsmall += 1
                        g = gpool.tile([P, c1 - c0, C], mybir.dt.bfloat16,
                                       tag=f"gat{ci}")
                        nc.gpsimd.dma_gather(
                            out_ap=g[:],
                            in_ap=src_half_aps[s],
                            idxs_ap=idx_t[:, seg * gh * 8 + c0 * 8:seg * gh * 8 + c1 * 8],
                            num_idxs=(c1 - c0) * P,
                            num_idxs_reg=(c1 - c0) * P,
                            elem_size=C,
                            queue_num=q,
                        )
                        parts.append((g, c0, c1))
                    gt.append(parts)

                acc = ppool.tile([P, C], mybir.dt.float32, space="PSUM")
                for gi in range(2 * gh):
                    s, gg = divmod(gi, gh)
                    g, c0, _ = next(p for p in gt[s] if p[1] <= gg < p[2])
                    nc.tensor.matmul(
                        out=acc[:],
                        lhsT=wt[:, gi * P:(gi + 1) * P],
                        rhs=g[:, gg - c0, :],
                        start=(gi == 0),
                        stop=(gi == 2 * gh - 1),
                    )
                ot = opool.tile([P, C], mybir.dt.float32, tag="out")
                nc.any.tensor_copy(out=ot[:], in_=acc[:])
                nc.sync.dma_start(out=out_d.ap()[j * P:(j + 1) * P, :], in_=ot[:])

    nc.compile()
    return nc


def kernel(H, edge_index, edge_weight, node_idx):
    H = np.asarray(H)
    edge_index = np.asarray(edge_index)
    edge_weight = np.ascontiguousarray(np.asarray(edge_weight), dtype=np.float32)
    node_idx = np.asarray(node_idx)

    inv = np.argsort(node_idx).astype(np.int64)  # id -> row (identity for arange)

    # ---- host bucketing: (core, dst-block, src-half) ----
    per_core = []   # (sloc_sorted, dloc_sorted, w_sorted, counts) per core
    gh = 1
    for b in range(B):
        dst = inv[edge_index[b, :, 0]]
        src = inv[edge_index[b, :, 1]]
        w = edge_weight[b]
        half = dst >= HALF0
        for h in (0, 1):
            m = half == (h == 1)
            d = dst[m] - h * HALF0
            s_rows = src[m]
            sh = s_rows >= SRC_SPLIT
            sloc = (s_rows - sh * SRC_SPLIT).astype(np.int16)
            bucket = (d >> 7) * 2 + sh
            order = np.argsort(bucket, kind="stable")
            bs = bucket[order]
            counts = np.bincount(bs, minlength=NSEG)
            gh = max(gh, int(np.ceil(counts.max() / P)))
            per_core.append((sloc[order], (d & 127)[order].astype(np.int64),
                             w[m][order], bs, counts))

    ngrp = NSEG * gh
    slots = ngrp * P
    in_maps = []
    for core in range(N_CORES):
        sloc, dloc, wv, bs, counts = per_core[core]
        starts = np.zeros(NSEG, np.int64)
        starts[1:] = np.cumsum(counts)[:-1]
        rank = np.arange(len(bs)) - starts[bs]
        slot = bs.astype(np.int64) * (gh * P) + rank

        sl = np.zeros(slots, np.int16)  # pads gather row 0 with w=0
        dl = np.zeros(slots, np.int64)
        wl = np.zeros(slots, np.float32)
        sl[slot] = sloc
        dl[slot] = dloc
        wl[slot] = wv

        # idx element e of segment k -> [e % 16, k*gh*8 + e//16], replicated x8
        idx16 = sl.reshape(NSEG, gh * 8, 16).transpose(2, 0, 1).reshape(16, NSEG * gh * 8)
        idx128 = np.ascontiguousarray(np.tile(idx16, (8, 1)))

        # one-hot weight tables: group k, edge-slot e (partition), dst col d:
        # W[e, k*128 + d] = w * (d == dloc).  Host-built, streamed per block.
        wtab = np.zeros((slots, P), np.float32)
        wtab[np.arange(slots), dl] = wl
        # reorder to [P partitions, ngrp*P cols]: partition e, col k*128+d
        wtab = np.ascontiguousarray(
            wtab.reshape(ngrp, P, P).transpose(1, 0, 2).reshape(P, ngrp * P)
        ).astype(ml_dtypes.bfloat16)

        in_maps.append({
            "h": np.ascontiguousarray(H[core // 2].reshape(N_NODES, C)).astype(
                ml_dtypes.bfloat16),
            "idx": idx128,
            "w": wtab,
        })

    global _last_in_maps
    _last_in_maps = in_maps
    nc = _prog_cache.get(gh)
    if nc is None:
        nc = _build_program(gh)
        _prog_cache[gh] = nc

    res = run_bass_kernel_spmd(nc, in_maps, list(range(N_CORES)))

    out = np.empty((B, N_NODES, HS, HS), np.float32)
    for b in range(B):
        r0 = res.results[2 * b]["out"]
        r1 = res.results[2 * b + 1]["out"]
        out[b, :HALF0] = r0.reshape(-1, HS, HS)
        out[b, HALF0:] = r1[:N_NODES - HALF0].reshape(-1, HS, HS)
    return out
